# revision 8
# baseline (speedup 1.0000x reference)
"""Local (Gaussian-banded) attention kernel for Trainium2, 8 NeuronCores. v2.

Math: out = rownorm(gauss_band(sigma)) @ (x @ Wg) @ Wout
Sigma in [0.5, 2.5] -> the Gaussian mask decays below fp32 resolution past
|i-j| > 32, so attention is a 65-tap banded matmul.

Sharding: core c = (batch b = c//2, seq-half s = c%2). s=1 halves are
row-reversed on host so the sequence edge is at local row 0 -> all 8 cores
run one program (pure SPMD).

v2 vs v1: all-bf16 operands (matmul cost is keyed on the moving operand's
dtype; bf16 streams 1 col/cycle at ANY width, f32r needs >=256-wide), which
lets stage 2 use 64-row output chunks whose 128-row source window aligns
with exactly one v tile: 1 matmul per (even chunk, head), 2 for odd chunks
via a partition-rolled band constant. Stage-2 PE cost drops 2x. Edge
normalization is baked into a chunk-0 band constant (no rescale op). The
PSUM->SBUF copies round-robin over DVE and Act instead of all-DVE, and a
warm-up matmul stream covers the PE DVFS ramp during the input DMAs.

Per-core pipeline (all matmuls bf16 x bf16 -> f32 PSUM):
  stage 1: v[1088,1024] = xpad @ Wg      (9 row tiles; lhsT = host xT)
  stage 2: attnT[dh=128, (j8,h8,128)] via 64-col band matmuls
  stage 3: out[1024,512] = attn @ Wout   (8 chunks, accumulate 8 heads)
"""

import sys

for _p in ("/opt/trn_rl_repo", "/root/.axon_site/_ro/trn_rl_repo"):
    if _p not in sys.path:
        sys.path.append(_p)

import numpy as np
import ml_dtypes

BF16 = ml_dtypes.bfloat16

B, N, D = 4, 2048, 512
H, DH = 8, 128
INNER = H * DH
W = 32                      # band half-width
VROWS = 1088                # 32 zero pad + 1024 own + 32 halo
NT = 9                      # v tiles: 8 x 128 + 1 x 64
NWARM = 67                  # PE warm-up matmuls (cover ~3us DVFS ramp)

_CACHE = {}


def _build_nc(debug_outputs=False):
    import concourse.mybir as mybir
    from concourse import bacc
    from concourse.tile import TileContext

    f32 = mybir.dt.float32
    bf = mybir.dt.bfloat16

    nc = bacc.Bacc(None, target_bir_lowering=False)

    xT = nc.dram_tensor("xT", [D, VROWS], bf, kind="ExternalInput")
    wg = nc.dram_tensor("Wg", [D, INNER], bf, kind="ExternalInput")
    wout = nc.dram_tensor("Wout", [INNER, D], bf, kind="ExternalInput")
    # bands: [128, (piece3, h8, 64)]: 0 = chunk-0 edge, 1 = interior,
    # 2 = partition-rolled interior (for the chunk-15 two-group tail path)
    bands = nc.dram_tensor("bands", [128, 3 * H * 64], bf, kind="ExternalInput")
    # bf16 output (host upcasts): halves the tail DMA; error budget is ample
    out = nc.dram_tensor("out", [1024, D], bf, kind="ExternalOutput")
    if debug_outputs:
        v_dbg = nc.dram_tensor("v_dbg", [NT * 128, INNER], bf,
                               kind="ExternalOutput")
        attnT_dbg = nc.dram_tensor("attnT_dbg", [128, 8 * H * 128], bf,
                                   kind="ExternalOutput")

    with TileContext(nc) as tc:
        with (
            tc.tile_pool(name="persist", bufs=1) as pp,
            tc.tile_pool(name="stage", bufs=2) as sp,
            tc.tile_pool(name="ps1", bufs=4, space="PSUM") as ps1,
            tc.tile_pool(name="ps2", bufs=2, space="PSUM") as ps2,
            tc.tile_pool(name="ps3", bufs=2, space="PSUM") as ps3,
        ):
            # ---- input DMAs, ordered so stage-1 starts early
            xT_sb = pp.tile([128, 4 * VROWS], bf, tag="xT", name="xT_sb")
            wg_sb = pp.tile([128, 4 * INNER], bf, tag="wg", name="wg_sb")
            xT_d = xT.rearrange("(k p) c -> p k c", p=128)
            xT_s = xT_sb.rearrange("p (k c) -> p k c", c=VROWS)
            wg_d = wg.rearrange("(k p) c -> p k c", p=128)
            wg_s = wg_sb.rearrange("p (k c) -> p k c", c=INNER)
            nc.sync.dma_start(out=xT_s[:, :, 0:256], in_=xT_d[:, :, 0:256])
            for k in range(4):
                nc.sync.dma_start(out=wg_s[:, k, :], in_=wg_d[:, k, :])
            nc.sync.dma_start(out=xT_s[:, :, 256:512], in_=xT_d[:, :, 256:512])
            nc.sync.dma_start(out=xT_s[:, :, 512:VROWS], in_=xT_d[:, :, 512:VROWS])
            bands_sb = pp.tile([128, 3 * H * 64], bf, tag="bands", name="bands_sb")
            nc.sync.dma_start(out=bands_sb, in_=bands[:, :])
            wout_sb = pp.tile([128, H * D], bf, tag="wout", name="wout_sb")
            nc.sync.dma_start(
                out=wout_sb.rearrange("p (h c) -> p h c", c=D),
                in_=wout.rearrange("(h p) c -> p h c", p=128))

            band0 = bands_sb[:, 0 * H * 64:1 * H * 64]
            bandE = bands_sb[:, 1 * H * 64:2 * H * 64]
            bandO = bands_sb[:, 2 * H * 64:3 * H * 64]

            # PE warm-up: the tensor engine DVFS-ramps over ~3us of
            # continuous execution; dummy matmuls on scratch data during the
            # input-DMA wait bring it to full clock before real work arrives.
            wsc = pp.tile([128, 192], bf, tag="wsc", name="wsc")
            nc.gpsimd.memset(wsc, 0.0)
            psW = ps3.tile([128, 512], f32, tag="s3", name="psW")
            for w in range(NWARM):
                nc.tensor.matmul(psW[:, 0:64], wsc[:, 0:128], wsc[:, 128:192],
                                 start=True, stop=True)

            v_sb = [pp.tile([128, INNER], bf, tag=f"v{t}", name=f"v{t}")
                    for t in range(NT)]
            # 64-row-shifted v tiles (padded rows [128u+64, 128u+192)) so odd
            # chunks use a single base-0 K=128 stationary (the PE rejects
            # mixed tile positions within one PSUM accumulation group)
            v_sh = [pp.tile([128, INNER], bf, tag=f"vs{u}", name=f"vs{u}")
                    for u in range(7)]
            # attnT layout: [dh(128), (j(8), h(8), r(128))]
            attnT = pp.tile([128, 8 * H * 128], bf, tag="attnT", name="attnT")
            at_r = attnT.rearrange("p (j h r) -> p j h r", j=8, h=H)

            # round-robin copy engines (GPSIMD cannot read PSUM)
            engs = [nc.vector, nc.scalar]
            ei = [0]

            def copy(dst, src):
                e = engs[ei[0] % 2]
                ei[0] += 1
                if e is nc.scalar:
                    e.copy(dst, src)
                else:
                    e.tensor_copy(dst, src)

            def stage1_finish(t, psA, psB):
                rows = 128 if t < 8 else 64
                if t >= 7:
                    # tail-critical: both halves in parallel on Act + DVE
                    nc.scalar.copy(v_sb[t][:rows, 0:512], psA[:rows, :])
                    nc.vector.tensor_copy(v_sb[t][:rows, 512:1024], psB[:rows, :])
                else:
                    copy(v_sb[t][:rows, 0:512], psA[:rows, :])
                    copy(v_sb[t][:rows, 512:1024], psB[:rows, :])
                if 1 <= t <= 7:
                    # SBUF->SBUF partition shift via HWDGE. v_sh[7] is
                    # not needed: chunk 15 takes the two-group tail path.
                    q = nc.sync
                    q.dma_start(out=v_sh[t - 1][0:64, :],
                                in_=v_sb[t - 1][64:128, :])
                    q.dma_start(out=v_sh[t - 1][64:128, :],
                                in_=v_sb[t][0:64, :])
                if debug_outputs:
                    nc.sync.dma_start(out=v_dbg[t * 128:t * 128 + rows, :],
                                      in_=v_sb[t][:rows, :])

            def stage1_mm(t, psA, psB, k):
                rows = 128 if t < 8 else 64
                lh = xT_sb[:, k * VROWS + t * 128: k * VROWS + t * 128 + rows]
                nc.tensor.matmul(psA[:rows, :], lh,
                                 wg_sb[:, k * INNER:k * INNER + 512],
                                 start=(k == 0), stop=(k == 3))
                nc.tensor.matmul(psB[:rows, :], lh,
                                 wg_sb[:, k * INNER + 512:(k + 1) * INNER],
                                 start=(k == 0), stop=(k == 3))

            def stage1(t):
                # tile 2 borrows ps3 (idle until stage3(0) at iter 3): after
                # pair01 holds all four ps1 banks, ps1 reuse would stall on
                # tile 0's copies
                pool, tg = (ps3, "s3") if t == 2 else (ps1, "s1")
                psA = pool.tile([128, 512], f32, tag=tg, name=f"psA{t}")
                psB = pool.tile([128, 512], f32, tag=tg, name=f"psB{t}")
                for k in range(4):
                    stage1_mm(t, psA, psB, k)
                stage1_finish(t, psA, psB)

            def stage1_pair01():
                # tiles 0+1 with k outermost: each per-k Wg DMA feeds 4
                # matmuls, smoothing the serialized-DMA startup
                pA0 = ps1.tile([128, 512], f32, tag="s1", name="psA0")
                pB0 = ps1.tile([128, 512], f32, tag="s1", name="psB0")
                pA1 = ps1.tile([128, 512], f32, tag="s1", name="psA1")
                pB1 = ps1.tile([128, 512], f32, tag="s1", name="psB1")
                for k in range(4):
                    stage1_mm(0, pA0, pB0, k)
                    stage1_mm(1, pA1, pB1, k)
                stage1_finish(0, pA0, pB0)
                stage1_finish(1, pA1, pB1)

            def stage2(i):
                # chunk i: attnT cols [64i, 64i+64) for all 8 heads
                ps = ps2.tile([128, 512], f32, tag="s2", name=f"ps2_{i}")
                src = v_sb[i // 2] if i % 2 == 0 else v_sh[(i - 1) // 2]
                bnd = band0 if i == 0 else bandE
                for h in range(H):
                    nc.tensor.matmul(
                        ps[:, h * 64:(h + 1) * 64],
                        src[:, h * 128:(h + 1) * 128],
                        bnd[:, h * 64:(h + 1) * 64],
                        start=(h == 0), stop=(h == H - 1))
                j, half = i // 2, i % 2
                dst = at_r[:, j, :, half * 64:half * 64 + 64]
                srcv = ps.rearrange("p (h r) -> p h r", r=64)
                if i == 13:
                    # stage3(6) gates on this copy: halve its latency by
                    # splitting across both engines
                    nc.scalar.copy(dst[:, 0:4], srcv[:, 0:4])
                    nc.vector.tensor_copy(dst[:, 4:8], srcv[:, 4:8])
                elif i == 14:
                    nc.scalar.copy(dst, srcv)   # keep DVE free for the
                else:                           # tail-critical chunk-15 add
                    copy(dst, srcv)

            # last chunk without v_sh: two uniform-tile-position PSUM groups
            # (v7 top rows at PE rows 64.., v8 bottom at 0..); psT lands in
            # attnT early via Act, then the DVE folds psB in from its single
            # allowed PSUM operand
            def stage2_15_top():
                psT = ps1.tile([128, 512], f32, tag="s1", name="ps15t")
                for h in range(H):
                    nc.tensor.matmul(
                        psT[:, h * 64:(h + 1) * 64],
                        v_sb[7][64:128, h * 128:(h + 1) * 128],
                        bandO[64:128, h * 64:(h + 1) * 64],
                        start=(h == 0), stop=(h == H - 1))
                nc.scalar.copy(at_r[:, 7, :, 64:128],
                               psT.rearrange("p (h r) -> p h r", r=64))

            def stage2_15_bot():
                psB_ = ps1.tile([128, 512], f32, tag="s1", name="ps15b")
                for h in range(H):
                    nc.tensor.matmul(
                        psB_[:, h * 64:(h + 1) * 64],
                        v_sb[8][0:64, h * 128:(h + 1) * 128],
                        bandO[0:64, h * 64:(h + 1) * 64],
                        start=(h == 0), stop=(h == H - 1))
                dst = at_r[:, 7, :, 64:128]
                nc.vector.tensor_add(
                    dst, dst, psB_.rearrange("p (h r) -> p h r", r=64))

            def stage3(j):
                # j=6 draws from ps1 (free after stage 1): with ps3's two
                # banks it would recycle stage3(4)'s bank before that copy
                # drains, stalling the PE at the tail
                pool, tg = (ps1, "s1") if j == 6 else (ps3, "s3")
                ps = pool.tile([128, 512], f32, tag=tg, name=f"ps3_{j}")
                for h in range(H):
                    nc.tensor.matmul(ps, at_r[:, j, h, :],
                                     wout_sb[:, h * D:(h + 1) * D],
                                     start=(h == 0), stop=(h == H - 1))
                otj = sp.tile([128, D], bf, tag="outt", name=f"ot{j}")
                if j == 6:
                    nc.scalar.copy(otj, ps)     # tail: keep DVE for the add
                else:
                    copy(otj, ps)
                nc.sync.dma_start(
                    out=out[j * 128:(j + 1) * 128, :], in_=otj)

            def stage3_last():
                # N-split the final chunk over two PSUM banks so half 1's
                # matmuls are independent of half 0's copy+DMA pipeline
                otj = sp.tile([128, D], bf, tag="outt", name="ot7")
                for half in range(2):
                    ps = ps3.tile([128, 256], f32, tag="s3", name=f"ps3_7{half}")
                    cs = slice(half * 256, half * 256 + 256)
                    for h in range(H):
                        nc.tensor.matmul(
                            ps, at_r[:, 7, h, :],
                            wout_sb[:, h * D + half * 256:h * D + half * 256 + 256],
                            start=(h == 0), stop=(h == H - 1))
                    # half 0 on DVE (free once the add retires), half 1 on Act
                    if half == 0:
                        nc.vector.tensor_copy(otj[:, cs], ps)
                    else:
                        nc.scalar.copy(otj[:, cs], ps)
                    nc.sync.dma_start(out=out[896:1024, cs], in_=otj[:, cs])

            # ---- software-pipelined emission:
            # iter t: stage1(t) | stage2 chunks of tile t-1 | stage3(t-3)
            stage1_pair01()
            for t in range(2, 8):
                stage1(t)
                if t == 6:
                    stage1(8)   # halo tile early: v8 copied well before tail
                if t == 2:
                    stage2(0)
                else:
                    stage2(2 * t - 4)          # even: tile t-2
                stage2(2 * t - 3)              # odd: tiles t-2, t-1
                if t >= 3:
                    stage3(t - 3)
            # tail: interleave so the chunk-15 add always has
            # add-independent PE work (st35/st36) running over it
            stage2_15_top()
            stage2(12)
            # stage3(5) split into two N=256 groups with chunk 13 wedged in
            # between: ch13's copy then drains under the second group and
            # stage3(6) is not left waiting on it
            ps5 = ps3.tile([128, 512], f32, tag="s3", name="ps3_5")
            for h in range(H):
                nc.tensor.matmul(ps5[:, 0:256], at_r[:, 5, h, :],
                                 wout_sb[:, h * D:h * D + 256],
                                 start=(h == 0), stop=(h == H - 1))
            stage2(13)
            for h in range(H):
                # start=False: has_written=0 sets on first write, preserving
                # the first group's region in the same bank
                nc.tensor.matmul(ps5[:, 256:512], at_r[:, 5, h, :],
                                 wout_sb[:, h * D + 256:(h + 1) * D],
                                 start=False, stop=(h == H - 1),
                                 skip_group_check=True)
            ot5 = sp.tile([128, D], bf, tag="outt", name="ot5")
            copy(ot5, ps5)
            nc.sync.dma_start(out=out[5 * 128:6 * 128, :], in_=ot5)
            stage2(14)
            stage2_15_bot()
            stage3(6)
            stage3(7)

            if debug_outputs:
                nc.sync.dma_start(out=attnT_dbg[:, :], in_=attnT[:, :])

    nc.compile()
    return nc


def _band_constants(sigma: np.ndarray):
    """band0/bandEven/bandOdd [128, H*64] bf16; interior + edge norms baked."""
    sig = np.asarray(sigma, np.float64).reshape(H)
    d = np.arange(W + 1, dtype=np.float64)
    wts = np.exp(-(d[None, :] ** 2) / (2.0 * sig[:, None] ** 2))  # [H, 33]
    tail = wts[:, 1:].sum(1)
    s_int = wts[:, 0] + 2.0 * tail
    # edge rowsum for out rows r=0..31 (left-truncated gaussian)
    re = np.arange(32)
    cum = np.concatenate([np.zeros((H, 1)), np.cumsum(wts[:, 1:], 1)], 1)
    s_edge = wts[:, [0]] + cum[:, np.minimum(re, W)] + tail[:, None]  # [H, 32]

    j = np.arange(128)
    r = np.arange(64)
    dist = np.abs(r[None, :] + 32 - j[:, None])          # [128, 64]
    msk = dist <= W
    g = np.where(msk[None], wts[:, np.minimum(dist, W).astype(int)], 0.0)  # [H,128,64]

    bandE = g / s_int[:, None, None]
    norm0 = np.concatenate([s_edge, np.broadcast_to(s_int[:, None], (H, 32))],
                           1)                             # [H, 64]
    band0 = g / norm0[:, None, :]
    band0 = np.where(j[None, :, None] < 32, 0.0, band0)   # zero the pad rows
    bandO = np.roll(bandE, 64, axis=1)

    def pack(b):  # [H, 128, 64] -> [128, H*64]
        return np.ascontiguousarray(
            b.transpose(1, 0, 2).reshape(128, H * 64)).astype(BF16)

    return np.ascontiguousarray(
        np.concatenate([pack(band0), pack(bandE), pack(bandO)], 1))


def _in_maps(x, Wg, Wout, sigma):
    bands = _band_constants(sigma)
    wg = np.ascontiguousarray(np.asarray(Wg, np.float32)).astype(BF16)
    wo = np.ascontiguousarray(np.asarray(Wout, np.float32)).astype(BF16)
    x = np.asarray(x, np.float32)
    maps = []
    for c in range(8):
        b, s = divmod(c, 2)
        z = x[b] if s == 0 else x[b, ::-1]
        xbuf = np.zeros((VROWS, D), np.float32)
        xbuf[32:] = z[:1056]
        maps.append({
            "xT": np.ascontiguousarray(xbuf.T).astype(BF16),
            "Wg": wg, "Wout": wo, "bands": bands,
        })
    return maps


def _get_nc():
    if "nc" not in _CACHE:
        _CACHE["nc"] = _build_nc()
    return _CACHE["nc"]


def run_spmd(in_maps, **kw):
    from concourse.bass_utils import run_bass_kernel_spmd
    return run_bass_kernel_spmd(_get_nc(), in_maps, core_ids=list(range(8)), **kw)


def _assemble(results):
    full = np.empty((B, N, D), np.float32)
    for c in range(8):
        b, s = divmod(c, 2)
        r = np.asarray(results[c]["out"], dtype=np.float32)
        if s == 0:
            full[b, :1024] = r
        else:
            full[b, 1024:] = r[::-1]
    return full


def kernel(x, Wg, Wout, sigma):
    maps = _in_maps(x, Wg, Wout, sigma)
    # the axon terminal occasionally reports a transient
    # NRT_EXEC_UNIT_UNRECOVERABLE on a cold run; a retry recovers it
    last = None
    for _ in range(3):
        try:
            res = run_spmd(maps)
            return _assemble(res.results)
        except Exception as e:
            last = e
    raise last


# revision 9
# speedup vs baseline: 1.0019x; 1.0019x over previous
"""Local (Gaussian-banded) attention kernel for Trainium2, 8 NeuronCores. v2.

Math: out = rownorm(gauss_band(sigma)) @ (x @ Wg) @ Wout
Sigma in [0.5, 2.5] -> the Gaussian mask decays below fp32 resolution past
|i-j| > 32, so attention is a 65-tap banded matmul.

Sharding: core c = (batch b = c//2, seq-half s = c%2). s=1 halves are
row-reversed on host so the sequence edge is at local row 0 -> all 8 cores
run one program (pure SPMD).

v2 vs v1: all-bf16 operands (matmul cost is keyed on the moving operand's
dtype; bf16 streams 1 col/cycle at ANY width, f32r needs >=256-wide), which
lets stage 2 use 64-row output chunks whose 128-row source window aligns
with exactly one v tile: 1 matmul per (even chunk, head), 2 for odd chunks
via a partition-rolled band constant. Stage-2 PE cost drops 2x. Edge
normalization is baked into a chunk-0 band constant (no rescale op). The
PSUM->SBUF copies round-robin over DVE and Act instead of all-DVE, and a
warm-up matmul stream covers the PE DVFS ramp during the input DMAs.

Per-core pipeline (all matmuls bf16 x bf16 -> f32 PSUM):
  stage 1: v[1088,1024] = xpad @ Wg      (9 row tiles; lhsT = host xT)
  stage 2: attnT[dh=128, (j8,h8,128)] via 64-col band matmuls
  stage 3: out[1024,512] = attn @ Wout   (8 chunks, accumulate 8 heads)
"""

import sys

for _p in ("/opt/trn_rl_repo", "/root/.axon_site/_ro/trn_rl_repo"):
    if _p not in sys.path:
        sys.path.append(_p)

import numpy as np
import ml_dtypes

BF16 = ml_dtypes.bfloat16

B, N, D = 4, 2048, 512
H, DH = 8, 128
INNER = H * DH
W = 32                      # band half-width
VROWS = 1088                # 32 zero pad + 1024 own + 32 halo
NT = 9                      # v tiles: 8 x 128 + 1 x 64
NWARM = 67                  # PE warm-up matmuls (cover ~3us DVFS ramp)

_CACHE = {}


def _build_nc(debug_outputs=False):
    import concourse.mybir as mybir
    from concourse import bacc
    from concourse.tile import TileContext

    f32 = mybir.dt.float32
    bf = mybir.dt.bfloat16

    nc = bacc.Bacc(None, target_bir_lowering=False)

    xT = nc.dram_tensor("xT", [D, VROWS], bf, kind="ExternalInput")
    wg = nc.dram_tensor("Wg", [D, INNER], bf, kind="ExternalInput")
    wout = nc.dram_tensor("Wout", [INNER, D], bf, kind="ExternalInput")
    # bands: [128, (piece3, h8, 64)]: 0 = chunk-0 edge, 1 = interior,
    # 2 = partition-rolled interior (for the chunk-15 two-group tail path)
    bands = nc.dram_tensor("bands", [128, 3 * H * 64], bf, kind="ExternalInput")
    # bf16 output (host upcasts): halves the tail DMA; error budget is ample
    out = nc.dram_tensor("out", [1024, D], bf, kind="ExternalOutput")
    if debug_outputs:
        v_dbg = nc.dram_tensor("v_dbg", [NT * 128, INNER], bf,
                               kind="ExternalOutput")
        attnT_dbg = nc.dram_tensor("attnT_dbg", [128, 8 * H * 128], bf,
                                   kind="ExternalOutput")

    with TileContext(nc) as tc:
        with (
            tc.tile_pool(name="persist", bufs=1) as pp,
            tc.tile_pool(name="stage", bufs=3) as sp,
            tc.tile_pool(name="ps1", bufs=4, space="PSUM") as ps1,
            tc.tile_pool(name="ps2", bufs=2, space="PSUM") as ps2,
            tc.tile_pool(name="ps3", bufs=2, space="PSUM") as ps3,
        ):
            # ---- input DMAs, ordered so stage-1 starts early
            xT_sb = pp.tile([128, 4 * VROWS], bf, tag="xT", name="xT_sb")
            wg_sb = pp.tile([128, 4 * INNER], bf, tag="wg", name="wg_sb")
            xT_d = xT.rearrange("(k p) c -> p k c", p=128)
            xT_s = xT_sb.rearrange("p (k c) -> p k c", c=VROWS)
            wg_d = wg.rearrange("(k p) c -> p k c", p=128)
            wg_s = wg_sb.rearrange("p (k c) -> p k c", c=INNER)
            nc.sync.dma_start(out=xT_s[:, :, 0:256], in_=xT_d[:, :, 0:256])
            for k in range(4):
                nc.sync.dma_start(out=wg_s[:, k, :], in_=wg_d[:, k, :])
            nc.sync.dma_start(out=xT_s[:, :, 256:512], in_=xT_d[:, :, 256:512])
            nc.sync.dma_start(out=xT_s[:, :, 512:VROWS], in_=xT_d[:, :, 512:VROWS])
            bands_sb = pp.tile([128, 3 * H * 64], bf, tag="bands", name="bands_sb")
            nc.sync.dma_start(out=bands_sb, in_=bands[:, :])
            wout_sb = pp.tile([128, H * D], bf, tag="wout", name="wout_sb")
            nc.sync.dma_start(
                out=wout_sb.rearrange("p (h c) -> p h c", c=D),
                in_=wout.rearrange("(h p) c -> p h c", p=128))

            band0 = bands_sb[:, 0 * H * 64:1 * H * 64]
            bandE = bands_sb[:, 1 * H * 64:2 * H * 64]
            bandO = bands_sb[:, 2 * H * 64:3 * H * 64]

            # PE warm-up: the tensor engine DVFS-ramps over ~3us of
            # continuous execution; dummy matmuls on scratch data during the
            # input-DMA wait bring it to full clock before real work arrives.
            wsc = pp.tile([128, 192], bf, tag="wsc", name="wsc")
            nc.gpsimd.memset(wsc, 0.0)
            psW = ps3.tile([128, 512], f32, tag="s3", name="psW")
            for w in range(NWARM):
                nc.tensor.matmul(psW[:, 0:64], wsc[:, 0:128], wsc[:, 128:192],
                                 start=True, stop=True)

            v_sb = [pp.tile([128, INNER], bf, tag=f"v{t}", name=f"v{t}")
                    for t in range(NT)]
            # 64-row-shifted v tiles (padded rows [128u+64, 128u+192)) so odd
            # chunks use a single base-0 K=128 stationary (the PE rejects
            # mixed tile positions within one PSUM accumulation group)
            v_sh = [pp.tile([128, INNER], bf, tag=f"vs{u}", name=f"vs{u}")
                    for u in range(7)]
            # attnT layout: [dh(128), (j(8), h(8), r(128))]
            attnT = pp.tile([128, 8 * H * 128], bf, tag="attnT", name="attnT")
            at_r = attnT.rearrange("p (j h r) -> p j h r", j=8, h=H)

            # round-robin copy engines (GPSIMD cannot read PSUM)
            engs = [nc.vector, nc.scalar]
            ei = [0]

            def copy(dst, src):
                e = engs[ei[0] % 2]
                ei[0] += 1
                if e is nc.scalar:
                    e.copy(dst, src)
                else:
                    e.tensor_copy(dst, src)

            def stage1_finish(t, psA, psB):
                rows = 128 if t < 8 else 64
                if t >= 7:
                    # tail-critical: both halves in parallel on Act + DVE
                    nc.scalar.copy(v_sb[t][:rows, 0:512], psA[:rows, :])
                    nc.vector.tensor_copy(v_sb[t][:rows, 512:1024], psB[:rows, :])
                else:
                    copy(v_sb[t][:rows, 0:512], psA[:rows, :])
                    copy(v_sb[t][:rows, 512:1024], psB[:rows, :])
                if 1 <= t <= 7:
                    # SBUF->SBUF partition shift via HWDGE. v_sh[7] is
                    # not needed: chunk 15 takes the two-group tail path.
                    q = nc.sync
                    q.dma_start(out=v_sh[t - 1][0:64, :],
                                in_=v_sb[t - 1][64:128, :])
                    q.dma_start(out=v_sh[t - 1][64:128, :],
                                in_=v_sb[t][0:64, :])
                if debug_outputs:
                    nc.sync.dma_start(out=v_dbg[t * 128:t * 128 + rows, :],
                                      in_=v_sb[t][:rows, :])

            def stage1_mm(t, psA, psB, k):
                rows = 128 if t < 8 else 64
                lh = xT_sb[:, k * VROWS + t * 128: k * VROWS + t * 128 + rows]
                nc.tensor.matmul(psA[:rows, :], lh,
                                 wg_sb[:, k * INNER:k * INNER + 512],
                                 start=(k == 0), stop=(k == 3))
                nc.tensor.matmul(psB[:rows, :], lh,
                                 wg_sb[:, k * INNER + 512:(k + 1) * INNER],
                                 start=(k == 0), stop=(k == 3))

            def stage1(t):
                # tile 2 borrows ps3 (idle until stage3(0) at iter 3): after
                # pair01 holds all four ps1 banks, ps1 reuse would stall on
                # tile 0's copies
                pool, tg = (ps3, "s3") if t == 2 else (ps1, "s1")
                psA = pool.tile([128, 512], f32, tag=tg, name=f"psA{t}")
                psB = pool.tile([128, 512], f32, tag=tg, name=f"psB{t}")
                for k in range(4):
                    stage1_mm(t, psA, psB, k)
                stage1_finish(t, psA, psB)

            def stage1_pair01():
                # tiles 0+1 with k outermost: each per-k Wg DMA feeds 4
                # matmuls, smoothing the serialized-DMA startup
                pA0 = ps1.tile([128, 512], f32, tag="s1", name="psA0")
                pB0 = ps1.tile([128, 512], f32, tag="s1", name="psB0")
                pA1 = ps1.tile([128, 512], f32, tag="s1", name="psA1")
                pB1 = ps1.tile([128, 512], f32, tag="s1", name="psB1")
                for k in range(4):
                    stage1_mm(0, pA0, pB0, k)
                    stage1_mm(1, pA1, pB1, k)
                stage1_finish(0, pA0, pB0)
                stage1_finish(1, pA1, pB1)

            def stage2(i):
                # chunk i: attnT cols [64i, 64i+64) for all 8 heads
                ps = ps2.tile([128, 512], f32, tag="s2", name=f"ps2_{i}")
                src = v_sb[i // 2] if i % 2 == 0 else v_sh[(i - 1) // 2]
                bnd = band0 if i == 0 else bandE
                for h in range(H):
                    nc.tensor.matmul(
                        ps[:, h * 64:(h + 1) * 64],
                        src[:, h * 128:(h + 1) * 128],
                        bnd[:, h * 64:(h + 1) * 64],
                        start=(h == 0), stop=(h == H - 1))
                j, half = i // 2, i % 2
                dst = at_r[:, j, :, half * 64:half * 64 + 64]
                srcv = ps.rearrange("p (h r) -> p h r", r=64)
                if i == 13:
                    # stage3(6) gates on this copy: halve its latency by
                    # splitting across both engines
                    nc.scalar.copy(dst[:, 0:4], srcv[:, 0:4])
                    nc.vector.tensor_copy(dst[:, 4:8], srcv[:, 4:8])
                elif i == 14:
                    nc.scalar.copy(dst, srcv)   # keep DVE free for the
                else:                           # tail-critical chunk-15 add
                    copy(dst, srcv)

            # last chunk without v_sh: two uniform-tile-position PSUM groups
            # (v7 top rows at PE rows 64.., v8 bottom at 0..); psT lands in
            # attnT early via Act, then the DVE folds psB in from its single
            # allowed PSUM operand
            def stage2_15_top():
                psT = ps1.tile([128, 512], f32, tag="s1", name="ps15t")
                for h in range(H):
                    nc.tensor.matmul(
                        psT[:, h * 64:(h + 1) * 64],
                        v_sb[7][64:128, h * 128:(h + 1) * 128],
                        bandO[64:128, h * 64:(h + 1) * 64],
                        start=(h == 0), stop=(h == H - 1))
                nc.scalar.copy(at_r[:, 7, :, 64:128],
                               psT.rearrange("p (h r) -> p h r", r=64))

            def stage2_15_bot():
                psB_ = ps1.tile([128, 512], f32, tag="s1", name="ps15b")
                for h in range(H):
                    nc.tensor.matmul(
                        psB_[:, h * 64:(h + 1) * 64],
                        v_sb[8][0:64, h * 128:(h + 1) * 128],
                        bandO[0:64, h * 64:(h + 1) * 64],
                        start=(h == 0), stop=(h == H - 1))
                dst = at_r[:, 7, :, 64:128]
                nc.vector.tensor_add(
                    dst, dst, psB_.rearrange("p (h r) -> p h r", r=64))

            def stage3(j):
                # j=6 draws from ps1 (free after stage 1): with ps3's two
                # banks it would recycle stage3(4)'s bank before that copy
                # drains, stalling the PE at the tail
                pool, tg = (ps1, "s1") if j == 6 else (ps3, "s3")
                ps = pool.tile([128, 512], f32, tag=tg, name=f"ps3_{j}")
                for h in range(H):
                    nc.tensor.matmul(ps, at_r[:, j, h, :],
                                     wout_sb[:, h * D:(h + 1) * D],
                                     start=(h == 0), stop=(h == H - 1))
                otj = sp.tile([128, D], bf, tag="outt", name=f"ot{j}")
                if j == 6:
                    nc.scalar.copy(otj, ps)     # tail: keep DVE for the add
                else:
                    copy(otj, ps)
                nc.sync.dma_start(
                    out=out[j * 128:(j + 1) * 128, :], in_=otj)

            def stage3_last():
                # N-split the final chunk over two PSUM banks so half 1's
                # matmuls are independent of half 0's copy+DMA pipeline
                otj = sp.tile([128, D], bf, tag="outt", name="ot7")
                for half in range(2):
                    ps = ps3.tile([128, 256], f32, tag="s3", name=f"ps3_7{half}")
                    cs = slice(half * 256, half * 256 + 256)
                    for h in range(H):
                        nc.tensor.matmul(
                            ps, at_r[:, 7, h, :],
                            wout_sb[:, h * D + half * 256:h * D + half * 256 + 256],
                            start=(h == 0), stop=(h == H - 1))
                    # half 0 on DVE (free once the add retires), half 1 on Act
                    if half == 0:
                        nc.vector.tensor_copy(otj[:, cs], ps)
                    else:
                        nc.scalar.copy(otj[:, cs], ps)
                    nc.sync.dma_start(out=out[896:1024, cs], in_=otj[:, cs])

            # ---- software-pipelined emission:
            # iter t: stage1(t) | stage2 chunks of tile t-1 | stage3(t-3)
            stage1_pair01()
            for t in range(2, 8):
                stage1(t)
                if t == 6:
                    stage1(8)   # halo tile early: v8 copied well before tail
                if t == 2:
                    stage2(0)
                else:
                    stage2(2 * t - 4)          # even: tile t-2
                stage2(2 * t - 3)              # odd: tiles t-2, t-1
                if t >= 3:
                    stage3(t - 3)
            # tail: interleave so the chunk-15 add always has
            # add-independent PE work (st35/st36) running over it
            stage2_15_top()
            stage2(12)
            # stage3(5) split into two N=256 groups with chunk 13 wedged in
            # between: ch13's copy then drains under the second group and
            # stage3(6) is not left waiting on it
            ps5 = ps3.tile([128, 512], f32, tag="s3", name="ps3_5")
            for h in range(H):
                nc.tensor.matmul(ps5[:, 0:256], at_r[:, 5, h, :],
                                 wout_sb[:, h * D:h * D + 256],
                                 start=(h == 0), stop=(h == H - 1))
            stage2(13)
            for h in range(H):
                # start=False: has_written=0 sets on first write, preserving
                # the first group's region in the same bank
                nc.tensor.matmul(ps5[:, 256:512], at_r[:, 5, h, :],
                                 wout_sb[:, h * D + 256:(h + 1) * D],
                                 start=False, stop=(h == H - 1),
                                 skip_group_check=True)
            ot5 = sp.tile([128, D], bf, tag="outt", name="ot5")
            copy(ot5, ps5)
            nc.sync.dma_start(out=out[5 * 128:6 * 128, :], in_=ot5)
            stage2(14)
            stage2_15_bot()
            stage3(6)
            stage3(7)

            if debug_outputs:
                nc.sync.dma_start(out=attnT_dbg[:, :], in_=attnT[:, :])

    nc.compile()
    return nc


def _band_constants(sigma: np.ndarray):
    """band0/bandEven/bandOdd [128, H*64] bf16; interior + edge norms baked."""
    sig = np.asarray(sigma, np.float64).reshape(H)
    d = np.arange(W + 1, dtype=np.float64)
    wts = np.exp(-(d[None, :] ** 2) / (2.0 * sig[:, None] ** 2))  # [H, 33]
    tail = wts[:, 1:].sum(1)
    s_int = wts[:, 0] + 2.0 * tail
    # edge rowsum for out rows r=0..31 (left-truncated gaussian)
    re = np.arange(32)
    cum = np.concatenate([np.zeros((H, 1)), np.cumsum(wts[:, 1:], 1)], 1)
    s_edge = wts[:, [0]] + cum[:, np.minimum(re, W)] + tail[:, None]  # [H, 32]

    j = np.arange(128)
    r = np.arange(64)
    dist = np.abs(r[None, :] + 32 - j[:, None])          # [128, 64]
    msk = dist <= W
    g = np.where(msk[None], wts[:, np.minimum(dist, W).astype(int)], 0.0)  # [H,128,64]

    bandE = g / s_int[:, None, None]
    norm0 = np.concatenate([s_edge, np.broadcast_to(s_int[:, None], (H, 32))],
                           1)                             # [H, 64]
    band0 = g / norm0[:, None, :]
    band0 = np.where(j[None, :, None] < 32, 0.0, band0)   # zero the pad rows
    bandO = np.roll(bandE, 64, axis=1)

    def pack(b):  # [H, 128, 64] -> [128, H*64]
        return np.ascontiguousarray(
            b.transpose(1, 0, 2).reshape(128, H * 64)).astype(BF16)

    return np.ascontiguousarray(
        np.concatenate([pack(band0), pack(bandE), pack(bandO)], 1))


def _in_maps(x, Wg, Wout, sigma):
    bands = _band_constants(sigma)
    wg = np.ascontiguousarray(np.asarray(Wg, np.float32)).astype(BF16)
    wo = np.ascontiguousarray(np.asarray(Wout, np.float32)).astype(BF16)
    x = np.asarray(x, np.float32)
    maps = []
    for c in range(8):
        b, s = divmod(c, 2)
        z = x[b] if s == 0 else x[b, ::-1]
        xbuf = np.zeros((VROWS, D), np.float32)
        xbuf[32:] = z[:1056]
        maps.append({
            "xT": np.ascontiguousarray(xbuf.T).astype(BF16),
            "Wg": wg, "Wout": wo, "bands": bands,
        })
    return maps


def _get_nc():
    if "nc" not in _CACHE:
        _CACHE["nc"] = _build_nc()
    return _CACHE["nc"]


def run_spmd(in_maps, **kw):
    from concourse.bass_utils import run_bass_kernel_spmd
    return run_bass_kernel_spmd(_get_nc(), in_maps, core_ids=list(range(8)), **kw)


def _assemble(results):
    full = np.empty((B, N, D), np.float32)
    for c in range(8):
        b, s = divmod(c, 2)
        r = np.asarray(results[c]["out"], dtype=np.float32)
        if s == 0:
            full[b, :1024] = r
        else:
            full[b, 1024:] = r[::-1]
    return full


def kernel(x, Wg, Wout, sigma):
    maps = _in_maps(x, Wg, Wout, sigma)
    # the axon terminal occasionally reports a transient
    # NRT_EXEC_UNIT_UNRECOVERABLE on a cold run; a retry recovers it
    last = None
    for _ in range(3):
        try:
            res = run_spmd(maps)
            return _assemble(res.results)
        except Exception as e:
            last = e
    raise last


# revision 10
# speedup vs baseline: 1.0022x; 1.0003x over previous
"""Local (Gaussian-banded) attention kernel for Trainium2, 8 NeuronCores. v2.

Math: out = rownorm(gauss_band(sigma)) @ (x @ Wg) @ Wout
Sigma in [0.5, 2.5] -> the Gaussian mask decays below fp32 resolution past
|i-j| > 32, so attention is a 65-tap banded matmul.

Sharding: core c = (batch b = c//2, seq-half s = c%2). s=1 halves are
row-reversed on host so the sequence edge is at local row 0 -> all 8 cores
run one program (pure SPMD).

v2 vs v1: all-bf16 operands (matmul cost is keyed on the moving operand's
dtype; bf16 streams 1 col/cycle at ANY width, f32r needs >=256-wide), which
lets stage 2 use 64-row output chunks whose 128-row source window aligns
with exactly one v tile: 1 matmul per (even chunk, head), 2 for odd chunks
via a partition-rolled band constant. Stage-2 PE cost drops 2x. Edge
normalization is baked into a chunk-0 band constant (no rescale op). The
PSUM->SBUF copies round-robin over DVE and Act instead of all-DVE, and a
warm-up matmul stream covers the PE DVFS ramp during the input DMAs.

Per-core pipeline (all matmuls bf16 x bf16 -> f32 PSUM):
  stage 1: v[1088,1024] = xpad @ Wg      (9 row tiles; lhsT = host xT)
  stage 2: attnT[dh=128, (j8,h8,128)] via 64-col band matmuls
  stage 3: out[1024,512] = attn @ Wout   (8 chunks, accumulate 8 heads)
"""

import sys

for _p in ("/opt/trn_rl_repo", "/root/.axon_site/_ro/trn_rl_repo"):
    if _p not in sys.path:
        sys.path.append(_p)

import numpy as np
import ml_dtypes

BF16 = ml_dtypes.bfloat16

B, N, D = 4, 2048, 512
H, DH = 8, 128
INNER = H * DH
W = 32                      # band half-width
VROWS = 1088                # 32 zero pad + 1024 own + 32 halo
NT = 9                      # v tiles: 8 x 128 + 1 x 64
NWARM = 67                  # PE warm-up matmuls (cover ~3us DVFS ramp)

_CACHE = {}


def _build_nc(debug_outputs=False):
    import concourse.mybir as mybir
    from concourse import bacc
    from concourse.tile import TileContext

    f32 = mybir.dt.float32
    bf = mybir.dt.bfloat16

    nc = bacc.Bacc(None, target_bir_lowering=False)

    xT = nc.dram_tensor("xT", [D, VROWS], bf, kind="ExternalInput")
    wg = nc.dram_tensor("Wg", [D, INNER], bf, kind="ExternalInput")
    wout = nc.dram_tensor("Wout", [INNER, D], bf, kind="ExternalInput")
    # bands: [128, (piece3, h8, 64)]: 0 = chunk-0 edge, 1 = interior,
    # 2 = partition-rolled interior (for the chunk-15 two-group tail path)
    bands = nc.dram_tensor("bands", [128, 3 * H * 64], bf, kind="ExternalInput")
    # bf16 output (host upcasts): halves the tail DMA; error budget is ample
    out = nc.dram_tensor("out", [1024, D], bf, kind="ExternalOutput")
    if debug_outputs:
        v_dbg = nc.dram_tensor("v_dbg", [NT * 128, INNER], bf,
                               kind="ExternalOutput")
        attnT_dbg = nc.dram_tensor("attnT_dbg", [128, 8 * H * 128], bf,
                                   kind="ExternalOutput")

    with TileContext(nc) as tc:
        with (
            tc.tile_pool(name="persist", bufs=1) as pp,
            tc.tile_pool(name="stage", bufs=3) as sp,
            tc.tile_pool(name="ps1", bufs=4, space="PSUM") as ps1,
            tc.tile_pool(name="ps2", bufs=2, space="PSUM") as ps2,
            tc.tile_pool(name="ps3", bufs=2, space="PSUM") as ps3,
        ):
            # ---- input DMAs, ordered so stage-1 starts early
            xT_sb = pp.tile([128, 4 * VROWS], bf, tag="xT", name="xT_sb")
            wg_sb = pp.tile([128, 4 * INNER], bf, tag="wg", name="wg_sb")
            xT_d = xT.rearrange("(k p) c -> p k c", p=128)
            xT_s = xT_sb.rearrange("p (k c) -> p k c", c=VROWS)
            wg_d = wg.rearrange("(k p) c -> p k c", p=128)
            wg_s = wg_sb.rearrange("p (k c) -> p k c", c=INNER)
            nc.sync.dma_start(out=xT_s[:, :, 0:256], in_=xT_d[:, :, 0:256])
            for k in range(4):
                nc.sync.dma_start(out=wg_s[:, k, :], in_=wg_d[:, k, :])
            nc.sync.dma_start(out=xT_s[:, :, 256:512], in_=xT_d[:, :, 256:512])
            nc.sync.dma_start(out=xT_s[:, :, 512:VROWS], in_=xT_d[:, :, 512:VROWS])
            bands_sb = pp.tile([128, 3 * H * 64], bf, tag="bands", name="bands_sb")
            nc.sync.dma_start(out=bands_sb, in_=bands[:, :])
            wout_sb = pp.tile([128, H * D], bf, tag="wout", name="wout_sb")
            nc.sync.dma_start(
                out=wout_sb.rearrange("p (h c) -> p h c", c=D),
                in_=wout.rearrange("(h p) c -> p h c", p=128))

            band0 = bands_sb[:, 0 * H * 64:1 * H * 64]
            bandE = bands_sb[:, 1 * H * 64:2 * H * 64]
            bandO = bands_sb[:, 2 * H * 64:3 * H * 64]

            # PE warm-up: the tensor engine DVFS-ramps over ~3us of
            # continuous execution; dummy matmuls on scratch data during the
            # input-DMA wait bring it to full clock before real work arrives.
            wsc = pp.tile([128, 192], bf, tag="wsc", name="wsc")
            nc.gpsimd.memset(wsc, 0.0)
            psW = ps3.tile([128, 512], f32, tag="s3", name="psW")
            for w in range(NWARM):
                nc.tensor.matmul(psW[:, 0:64], wsc[:, 0:128], wsc[:, 128:192],
                                 start=True, stop=True)

            v_sb = [pp.tile([128, INNER], bf, tag=f"v{t}", name=f"v{t}")
                    for t in range(NT)]
            # 64-row-shifted v tiles (padded rows [128u+64, 128u+192)) so odd
            # chunks use a single base-0 K=128 stationary (the PE rejects
            # mixed tile positions within one PSUM accumulation group)
            v_sh = [pp.tile([128, INNER], bf, tag=f"vs{u}", name=f"vs{u}")
                    for u in range(7)]
            # attnT layout: [dh(128), (j(8), h(8), r(128))]
            attnT = pp.tile([128, 8 * H * 128], bf, tag="attnT", name="attnT")
            at_r = attnT.rearrange("p (j h r) -> p j h r", j=8, h=H)

            # round-robin copy engines (GPSIMD cannot read PSUM)
            engs = [nc.vector, nc.scalar]
            ei = [0]

            def copy(dst, src):
                e = engs[ei[0] % 2]
                ei[0] += 1
                if e is nc.scalar:
                    e.copy(dst, src)
                else:
                    e.tensor_copy(dst, src)

            def stage1_finish(t, psA, psB):
                rows = 128 if t < 8 else 64
                if t >= 7:
                    # tail-critical: both halves in parallel on Act + DVE
                    nc.scalar.copy(v_sb[t][:rows, 0:512], psA[:rows, :])
                    nc.vector.tensor_copy(v_sb[t][:rows, 512:1024], psB[:rows, :])
                else:
                    copy(v_sb[t][:rows, 0:512], psA[:rows, :])
                    copy(v_sb[t][:rows, 512:1024], psB[:rows, :])
                if 1 <= t <= 7:
                    # SBUF->SBUF partition shift via HWDGE. v_sh[7] is
                    # not needed: chunk 15 takes the two-group tail path.
                    q = nc.sync
                    q.dma_start(out=v_sh[t - 1][0:64, :],
                                in_=v_sb[t - 1][64:128, :])
                    q.dma_start(out=v_sh[t - 1][64:128, :],
                                in_=v_sb[t][0:64, :])
                if debug_outputs:
                    nc.sync.dma_start(out=v_dbg[t * 128:t * 128 + rows, :],
                                      in_=v_sb[t][:rows, :])

            def stage1_mm(t, psA, psB, k):
                rows = 128 if t < 8 else 64
                lh = xT_sb[:, k * VROWS + t * 128: k * VROWS + t * 128 + rows]
                nc.tensor.matmul(psA[:rows, :], lh,
                                 wg_sb[:, k * INNER:k * INNER + 512],
                                 start=(k == 0), stop=(k == 3))
                nc.tensor.matmul(psB[:rows, :], lh,
                                 wg_sb[:, k * INNER + 512:(k + 1) * INNER],
                                 start=(k == 0), stop=(k == 3))

            def stage1(t):
                # tile 2 borrows ps3 (idle until stage3(0) at iter 3): after
                # pair01 holds all four ps1 banks, ps1 reuse would stall on
                # tile 0's copies
                pool, tg = (ps3, "s3") if t == 2 else (ps1, "s1")
                psA = pool.tile([128, 512], f32, tag=tg, name=f"psA{t}")
                psB = pool.tile([128, 512], f32, tag=tg, name=f"psB{t}")
                for k in range(4):
                    stage1_mm(t, psA, psB, k)
                stage1_finish(t, psA, psB)

            def stage1_pair01():
                # tiles 0+1 with k outermost: each per-k Wg DMA feeds 4
                # matmuls, smoothing the serialized-DMA startup
                pA0 = ps1.tile([128, 512], f32, tag="s1", name="psA0")
                pB0 = ps1.tile([128, 512], f32, tag="s1", name="psB0")
                pA1 = ps1.tile([128, 512], f32, tag="s1", name="psA1")
                pB1 = ps1.tile([128, 512], f32, tag="s1", name="psB1")
                for k in range(4):
                    stage1_mm(0, pA0, pB0, k)
                    stage1_mm(1, pA1, pB1, k)
                stage1_finish(0, pA0, pB0)
                stage1_finish(1, pA1, pB1)

            def stage2(i):
                # chunk i: attnT cols [64i, 64i+64) for all 8 heads
                ps = ps2.tile([128, 512], f32, tag="s2", name=f"ps2_{i}")
                src = v_sb[i // 2] if i % 2 == 0 else v_sh[(i - 1) // 2]
                bnd = band0 if i == 0 else bandE
                for h in range(H):
                    nc.tensor.matmul(
                        ps[:, h * 64:(h + 1) * 64],
                        src[:, h * 128:(h + 1) * 128],
                        bnd[:, h * 64:(h + 1) * 64],
                        start=(h == 0), stop=(h == H - 1))
                j, half = i // 2, i % 2
                dst = at_r[:, j, :, half * 64:half * 64 + 64]
                srcv = ps.rearrange("p (h r) -> p h r", r=64)
                if i in (1, 13):
                    # stage3(0)/(6) gate on these copies: halve the latency
                    # by splitting across both engines
                    nc.scalar.copy(dst[:, 0:4], srcv[:, 0:4])
                    nc.vector.tensor_copy(dst[:, 4:8], srcv[:, 4:8])
                elif i == 14:
                    nc.scalar.copy(dst, srcv)   # keep DVE free for the
                else:                           # tail-critical chunk-15 add
                    copy(dst, srcv)

            # last chunk without v_sh: two uniform-tile-position PSUM groups
            # (v7 top rows at PE rows 64.., v8 bottom at 0..); psT lands in
            # attnT early via Act, then the DVE folds psB in from its single
            # allowed PSUM operand
            def stage2_15_top():
                psT = ps1.tile([128, 512], f32, tag="s1", name="ps15t")
                for h in range(H):
                    nc.tensor.matmul(
                        psT[:, h * 64:(h + 1) * 64],
                        v_sb[7][64:128, h * 128:(h + 1) * 128],
                        bandO[64:128, h * 64:(h + 1) * 64],
                        start=(h == 0), stop=(h == H - 1))
                nc.scalar.copy(at_r[:, 7, :, 64:128],
                               psT.rearrange("p (h r) -> p h r", r=64))

            def stage2_15_bot():
                psB_ = ps1.tile([128, 512], f32, tag="s1", name="ps15b")
                for h in range(H):
                    nc.tensor.matmul(
                        psB_[:, h * 64:(h + 1) * 64],
                        v_sb[8][0:64, h * 128:(h + 1) * 128],
                        bandO[0:64, h * 64:(h + 1) * 64],
                        start=(h == 0), stop=(h == H - 1))
                dst = at_r[:, 7, :, 64:128]
                nc.vector.tensor_add(
                    dst, dst, psB_.rearrange("p (h r) -> p h r", r=64))

            def stage3(j):
                # j=6 draws from ps1 (free after stage 1): with ps3's two
                # banks it would recycle stage3(4)'s bank before that copy
                # drains, stalling the PE at the tail
                pool, tg = (ps1, "s1") if j == 6 else (ps3, "s3")
                ps = pool.tile([128, 512], f32, tag=tg, name=f"ps3_{j}")
                for h in range(H):
                    nc.tensor.matmul(ps, at_r[:, j, h, :],
                                     wout_sb[:, h * D:(h + 1) * D],
                                     start=(h == 0), stop=(h == H - 1))
                otj = sp.tile([128, D], bf, tag="outt", name=f"ot{j}")
                if j == 6:
                    nc.scalar.copy(otj, ps)     # tail: keep DVE for the add
                else:
                    copy(otj, ps)
                nc.sync.dma_start(
                    out=out[j * 128:(j + 1) * 128, :], in_=otj)

            def stage3_last():
                # N-split the final chunk over two PSUM banks so half 1's
                # matmuls are independent of half 0's copy+DMA pipeline
                otj = sp.tile([128, D], bf, tag="outt", name="ot7")
                for half in range(2):
                    ps = ps3.tile([128, 256], f32, tag="s3", name=f"ps3_7{half}")
                    cs = slice(half * 256, half * 256 + 256)
                    for h in range(H):
                        nc.tensor.matmul(
                            ps, at_r[:, 7, h, :],
                            wout_sb[:, h * D + half * 256:h * D + half * 256 + 256],
                            start=(h == 0), stop=(h == H - 1))
                    # half 0 on DVE (free once the add retires), half 1 on Act
                    if half == 0:
                        nc.vector.tensor_copy(otj[:, cs], ps)
                    else:
                        nc.scalar.copy(otj[:, cs], ps)
                    nc.sync.dma_start(out=out[896:1024, cs], in_=otj[:, cs])

            # ---- software-pipelined emission:
            # iter t: stage1(t) | stage2 chunks of tile t-1 | stage3(t-3)
            stage1_pair01()
            for t in range(2, 8):
                stage1(t)
                if t == 6:
                    stage1(8)   # halo tile early: v8 copied well before tail
                if t == 2:
                    stage2(0)
                else:
                    stage2(2 * t - 4)          # even: tile t-2
                stage2(2 * t - 3)              # odd: tiles t-2, t-1
                if t >= 3:
                    stage3(t - 3)
            # tail: interleave so the chunk-15 add always has
            # add-independent PE work (st35/st36) running over it
            stage2_15_top()
            stage2(12)
            # stage3(5) split into two N=256 groups with chunk 13 wedged in
            # between: ch13's copy then drains under the second group and
            # stage3(6) is not left waiting on it
            ps5 = ps3.tile([128, 512], f32, tag="s3", name="ps3_5")
            for h in range(H):
                nc.tensor.matmul(ps5[:, 0:256], at_r[:, 5, h, :],
                                 wout_sb[:, h * D:h * D + 256],
                                 start=(h == 0), stop=(h == H - 1))
            stage2(13)
            for h in range(H):
                # start=False: has_written=0 sets on first write, preserving
                # the first group's region in the same bank
                nc.tensor.matmul(ps5[:, 256:512], at_r[:, 5, h, :],
                                 wout_sb[:, h * D + 256:(h + 1) * D],
                                 start=False, stop=(h == H - 1),
                                 skip_group_check=True)
            ot5 = sp.tile([128, D], bf, tag="outt", name="ot5")
            copy(ot5, ps5)
            nc.sync.dma_start(out=out[5 * 128:6 * 128, :], in_=ot5)
            stage2(14)
            stage2_15_bot()
            stage3(6)
            stage3(7)

            if debug_outputs:
                nc.sync.dma_start(out=attnT_dbg[:, :], in_=attnT[:, :])

    nc.compile()
    return nc


def _band_constants(sigma: np.ndarray):
    """band0/bandEven/bandOdd [128, H*64] bf16; interior + edge norms baked."""
    sig = np.asarray(sigma, np.float64).reshape(H)
    d = np.arange(W + 1, dtype=np.float64)
    wts = np.exp(-(d[None, :] ** 2) / (2.0 * sig[:, None] ** 2))  # [H, 33]
    tail = wts[:, 1:].sum(1)
    s_int = wts[:, 0] + 2.0 * tail
    # edge rowsum for out rows r=0..31 (left-truncated gaussian)
    re = np.arange(32)
    cum = np.concatenate([np.zeros((H, 1)), np.cumsum(wts[:, 1:], 1)], 1)
    s_edge = wts[:, [0]] + cum[:, np.minimum(re, W)] + tail[:, None]  # [H, 32]

    j = np.arange(128)
    r = np.arange(64)
    dist = np.abs(r[None, :] + 32 - j[:, None])          # [128, 64]
    msk = dist <= W
    g = np.where(msk[None], wts[:, np.minimum(dist, W).astype(int)], 0.0)  # [H,128,64]

    bandE = g / s_int[:, None, None]
    norm0 = np.concatenate([s_edge, np.broadcast_to(s_int[:, None], (H, 32))],
                           1)                             # [H, 64]
    band0 = g / norm0[:, None, :]
    band0 = np.where(j[None, :, None] < 32, 0.0, band0)   # zero the pad rows
    bandO = np.roll(bandE, 64, axis=1)

    def pack(b):  # [H, 128, 64] -> [128, H*64]
        return np.ascontiguousarray(
            b.transpose(1, 0, 2).reshape(128, H * 64)).astype(BF16)

    return np.ascontiguousarray(
        np.concatenate([pack(band0), pack(bandE), pack(bandO)], 1))


def _in_maps(x, Wg, Wout, sigma):
    bands = _band_constants(sigma)
    wg = np.ascontiguousarray(np.asarray(Wg, np.float32)).astype(BF16)
    wo = np.ascontiguousarray(np.asarray(Wout, np.float32)).astype(BF16)
    x = np.asarray(x, np.float32)
    maps = []
    for c in range(8):
        b, s = divmod(c, 2)
        z = x[b] if s == 0 else x[b, ::-1]
        xbuf = np.zeros((VROWS, D), np.float32)
        xbuf[32:] = z[:1056]
        maps.append({
            "xT": np.ascontiguousarray(xbuf.T).astype(BF16),
            "Wg": wg, "Wout": wo, "bands": bands,
        })
    return maps


def _get_nc():
    if "nc" not in _CACHE:
        _CACHE["nc"] = _build_nc()
    return _CACHE["nc"]


def run_spmd(in_maps, **kw):
    from concourse.bass_utils import run_bass_kernel_spmd
    return run_bass_kernel_spmd(_get_nc(), in_maps, core_ids=list(range(8)), **kw)


def _assemble(results):
    full = np.empty((B, N, D), np.float32)
    for c in range(8):
        b, s = divmod(c, 2)
        r = np.asarray(results[c]["out"], dtype=np.float32)
        if s == 0:
            full[b, :1024] = r
        else:
            full[b, 1024:] = r[::-1]
    return full


def kernel(x, Wg, Wout, sigma):
    maps = _in_maps(x, Wg, Wout, sigma)
    # the axon terminal occasionally reports a transient
    # NRT_EXEC_UNIT_UNRECOVERABLE on a cold run; a retry recovers it
    last = None
    for _ in range(3):
        try:
            res = run_spmd(maps)
            return _assemble(res.results)
        except Exception as e:
            last = e
    raise last
